# revision 51
# baseline (speedup 1.0000x reference)
"""AdvancedVectorMemory fused kernel for 8 Trainium2 NeuronCores.

Sharding: core c handles batch b = c//4 and heads 4*(c%4)..4*(c%4)+3
(data parallel over batch, tensor parallel over heads). Attention runs
per head with fused denominators (ones column in V), s-chunk-outer so a
group-8 Shared-output AllGather per s-chunk (bf16 payload) redistributes
retrieved values early. The epilogue (Wo, gate MLP, residual) is sharded
4-ways by s-columns within every chunk and woven into the attention
phase; only the last 256 columns run as tail.

Attention AV runs as fp8e4m3 DoubleRow matmuls (256-wide contraction per
pass); QK and projections run bf16. The scalar engine's exp is the
critical resource; issue order keeps it fed.
"""
import sys
import numpy as np

for _p in ('/opt/trn_rl_repo', '/root/.axon_site/_ro/trn_rl_repo'):
    if _p not in sys.path:
        sys.path.insert(0, _p)

B, S, M = 2, 2048, 4096
DM, DK = 1024, 768
H, Dh = 16, 64
NC = 8
GS = 4            # cores per batch group
SC_W = 512        # s-chunk width
N_SC = S // SC_W  # 4 s-chunks
N_MC = 8          # m blocks of 512
N_TP = 16         # m tile-pairs of 256
EC = 128          # epilogue columns per core per s-chunk

_PROG = None


def _build_program():
    from concourse import bacc, mybir, tile
    import concourse.bass as bass

    F32 = mybir.dt.float32
    F32R = mybir.dt.float32r
    BF16 = mybir.dt.bfloat16
    FP8 = mybir.dt.float8e4
    I32 = mybir.dt.int32
    AF = mybir.ActivationFunctionType
    ALU = mybir.AluOpType
    DR = mybir.MatmulPerfMode.DoubleRow

    nc = bacc.Bacc('TRN2', target_bir_lowering=False, debug=False, num_devices=NC)

    def din(name, shape, dt=BF16):
        return nc.dram_tensor(name, shape, dt, kind='ExternalInput').ap()

    qT = din('qT', [DM, S], FP8)
    mkT = din('mkT', [DK, M], FP8)
    mvT = din('mvT', [DK, M], FP8)
    wqT = din('wqT', [DM, 256], FP8)
    wkT = din('wkT', [DK, 256], FP8)
    wvT = din('wvT', [DK, 256], FP8)
    woT = din('woT', [DM, DM])
    wg1T = din('wg1T', [2 * DM, DM])
    wg2T = din('wg2T', [DM, 2])
    qsT = din('qsT', [DM, 512])
    bqv = din('bqv', [2, 128], F32)
    bkv = din('bkv', [2, 128], F32)
    bo2v = din('bo2v', [8, 128], F32)
    bg1v = din('bg1v', [8, 128], F32)
    bg2v = din('bg2v', [2, 1], F32)
    sel16 = din('sel16', [16, 1024], F32R)
    bc0 = din('bc0', [2, 128], F32R)
    vones = din('vones', [128, 32], FP8)
    gidx = din('gidx', [8, 128], I32)
    didx = din('didx', [1, 16], I32)

    out_t = nc.dram_tensor('out_t', [DM, 512], F32, kind='ExternalOutput').ap()

    with tile.TileContext(nc) as tc:
        with tc.tile_pool(name='consts', bufs=1) as consts, \
             tc.tile_pool(name='dram', bufs=1, space='DRAM') as dram:

            # ---------------- constants / weights ----------------
            wq_sb = consts.tile([128, 2048], FP8, tag='wq_sb')
            for k in range(8):
                nc.sync.dma_start(out=wq_sb[:, 256 * k:256 * (k + 1)],
                                  in_=wqT[128 * k:128 * (k + 1), :])
            wk_sb = consts.tile([128, 1536], FP8, tag='wk_sb')
            wv_sb = consts.tile([128, 1536], FP8, tag='wv_sb')
            for k in range(6):
                nc.sync.dma_start(out=wk_sb[:, 256 * k:256 * (k + 1)],
                                  in_=wkT[128 * k:128 * (k + 1), :])
                nc.sync.dma_start(out=wv_sb[:, 256 * k:256 * (k + 1)],
                                  in_=wvT[128 * k:128 * (k + 1), :])
            bq_sb = consts.tile([128, 2], F32, tag='bq_sb')
            bk_sb = consts.tile([128, 2], F32, tag='bk_sb')
            for p in range(2):
                nc.scalar.dma_start(out=bq_sb[:, p:p + 1], in_=bqv[p:p + 1, :])
                nc.scalar.dma_start(out=bk_sb[:, p:p + 1], in_=bkv[p:p + 1, :])
            bo2_sb = consts.tile([128, 8], F32, tag='bo2_sb')
            bg1_sb = consts.tile([128, 8], F32, tag='bg1_sb')
            for k in range(8):
                nc.scalar.dma_start(out=bo2_sb[:, k:k + 1], in_=bo2v[k:k + 1, :])
                nc.scalar.dma_start(out=bg1_sb[:, k:k + 1], in_=bg1v[k:k + 1, :])
            bg2_sb = consts.tile([2, 1], F32, tag='bg2_sb')
            nc.scalar.dma_start(out=bg2_sb[:], in_=bg2v[:])
            sel16_sb = consts.tile([16, 1024], F32R, tag='sel16_sb')
            nc.scalar.dma_start(out=sel16_sb[:], in_=sel16[:])
            bc0_sb = consts.tile([2, 128], F32R, tag='bc0_sb')
            nc.scalar.dma_start(out=bc0_sb[:], in_=bc0[:])
            vones_sb = consts.tile([128, 32], FP8, tag='vones_sb')
            nc.scalar.dma_start(out=vones_sb[:], in_=vones[:])
            gidx_sb = []
            for kc in range(8):
                gt = consts.tile([128, 1], I32, tag=f'gidx{kc}', name=f'gidx{kc}')
                nc.scalar.dma_start(out=gt[:], in_=gidx[kc:kc + 1, :])
                gidx_sb.append(gt)
            didx_sb = consts.tile([16, 1], I32, tag='didx_sb')
            nc.scalar.dma_start(out=didx_sb[:], in_=didx[0:1, :])
            wg2_sb = consts.tile([128, 16], BF16, tag='wg2_sb')
            for k in range(8):
                nc.scalar.dma_start(out=wg2_sb[:, 2 * k:2 * (k + 1)],
                                    in_=wg2T[128 * k:128 * (k + 1), :])
            # epilogue weights are loaded late (after q/mk/mv DMAs are queued)
            wo_sb = consts.tile([128, 8 * DM], BF16, tag='wo_sb')
            wg1_sb = consts.tile([128, 16 * DM], BF16, tag='wg1_sb')
            qs_sb = consts.tile([128, 8 * 512], BF16, tag='qs_sb')

            def load_epilogue_weights():
                for k in range(8):
                    nc.scalar.dma_start(out=wo_sb[:, DM * k:DM * (k + 1)],
                                        in_=woT[128 * k:128 * (k + 1), :])
                for k in range(16):
                    nc.scalar.dma_start(out=wg1_sb[:, DM * k:DM * (k + 1)],
                                        in_=wg1T[128 * k:128 * (k + 1), :])
                for k in range(8):
                    nc.scalar.dma_start(out=qs_sb[:, 512 * k:512 * (k + 1)],
                                        in_=qsT[128 * k:128 * (k + 1), :])

            # collective buffers, one per (s-chunk, head-pair)
            rt_in = {(s, p): dram.tile([132, 512], BF16, tag=f'rt_in{s}{p}',
                                       name=f'rt_in{s}{p}')
                     for s in range(N_SC) for p in range(2)}
            rt_out = {(s, p): dram.tile([4224, 128], BF16, tag=f'rt_out{s}{p}',
                                        name=f'rt_out{s}{p}', addr_space='Shared')
                      for s in range(N_SC) for p in range(2)}

            # ---------------- persistent compute tiles ----------------
            main = consts  # same lifetime
            # Q operand stored per (pair, head-in-pair) with the other
            # head's 64 rows zeroed, so QK runs as full 128-contraction
            # matmuls (1 cyc/row) instead of half-array 64-contraction
            # (2 cyc/row)
            qt_z = [[main.tile([128, S], BF16, tag=f'qt_z{p}{a}',
                               name=f'qt_z{p}{a}') for a in range(2)]
                    for p in range(2)]
            kt_sb = [[main.tile([128, 512], BF16, tag=f'kt{p}_{mc}',
                                name=f'kt{p}_{mc}') for mc in range(N_MC)]
                     for p in range(2)]
            v2_sb = [main.tile([128, 544], FP8, tag=f'v2_{tp}', name=f'v2_{tp}')
                     for tp in range(N_TP)]
            oT = main.tile([128, 8 * 512], BF16, tag='oT')
            sl = main.tile([128, 8 * 512], BF16, tag='sl')
            pgq_sb = main.tile([128, 8 * 512], BF16, tag='pgq_sb')

            # ones columns of the fp8 V tiles (positions 64..67 per half/head)
            for tp in range(N_TP):
                v4 = v2_sb[tp].rearrange('p (k h c) -> p k h c', k=2, h=4)
                nc.vector.tensor_copy(
                    v4[:, :, :, 64:68],
                    vones_sb[:].rearrange('p (k h c) -> p k h c', k=2, h=4))
            # zero the dead head rows of the Q operands
            for p in range(2):
                for a in range(2):
                    nc.vector.memset(qt_z[p][a][64 * (1 - a):64 * (2 - a), :], 0.0)

            # Q input chunks stay resident (fp8) so the projection can run
            # per-s-chunk: only chunk j=0 gates the start of attention
            qtc_sb = []
            for k in range(8):
                qc = consts.tile([128, S], FP8, tag=f'qtc{k}', name=f'qtc{k}')
                nc.sync.dma_start(out=qc[:], in_=qT[128 * k:128 * (k + 1), :])
                qtc_sb.append(qc)

            # ---------------- weave: KV proj + attention + epilogue ----
            ctx_mkin = tc.tile_pool(name='mkin', bufs=3)
            mkin = ctx_mkin.__enter__()
            ctx_psKV = tc.tile_pool(name='psKV', bufs=1, space='PSUM')
            psKV = ctx_psKV.__enter__()

            with tc.tile_pool(name='psQK', bufs=2, space='PSUM') as psQK, \
                 tc.tile_pool(name='psAV', bufs=1, space='PSUM') as psAV, \
                 tc.tile_pool(name='attn', bufs=4) as apool, \
                 tc.tile_pool(name='rtst', bufs=2) as rtst, \
                 tc.tile_pool(name='ep', bufs=2) as ept, \
                 tc.tile_pool(name='rtn8', bufs=1) as rtnp:

                def qproj_j(j):
                    for p in range(2):
                        pq = psKV.tile([128, 512], F32, tag='pk')
                        for k in range(8):
                            nc.tensor.matmul(
                                pq[:],
                                wq_sb[:, 256 * k + 128 * p:256 * k + 128 * (p + 1)],
                                qtc_sb[k][:, 512 * j:512 * (j + 1)],
                                start=(k == 0), stop=(k == 7))
                        for a in range(2):
                            nc.vector.tensor_scalar_add(
                                qt_z[p][a][64 * a:64 * (a + 1),
                                           512 * j:512 * (j + 1)],
                                pq[64 * a:64 * (a + 1), :],
                                bq_sb[64 * a:64 * (a + 1), p:p + 1])

                kvb = {}

                def kv_dma(mc):
                    mkb = mkin.tile([128, 3072], FP8, tag='mkb')
                    mvb = mkin.tile([128, 3072], FP8, tag='mvb')
                    for k in range(6):
                        nc.sync.dma_start(
                            out=mkb[:, 512 * k:512 * (k + 1)],
                            in_=mkT[128 * k:128 * (k + 1), 512 * mc:512 * (mc + 1)])
                        nc.sync.dma_start(
                            out=mvb[:, 512 * k:512 * (k + 1)],
                            in_=mvT[128 * k:128 * (k + 1), 512 * mc:512 * (mc + 1)])
                    kvb[mc] = (mkb, mvb)

                def kv_block(mc):
                    mkb, mvb = kvb.pop(mc)
                    for p in range(2):
                        pk = psKV.tile([128, 512], F32, tag='pk')
                        for k in range(6):
                            nc.tensor.matmul(
                                pk[:],
                                wk_sb[:, 256 * k + 128 * p:256 * k + 128 * (p + 1)],
                                mkb[:, 512 * k:512 * (k + 1)],
                                start=(k == 0), stop=(k == 5))
                        nc.vector.tensor_scalar_add(
                            kt_sb[p][mc][:], pk[:], bk_sb[:, p:p + 1])
                    for ml in range(4):
                        mt = 4 * mc + ml
                        tp, half = mt // 2, mt % 2
                        pv = psKV.tile([128, 256], F32, tag='pv')
                        for k in range(6):
                            nc.tensor.matmul(
                                pv[:],
                                mvb[:, 512 * k + 128 * ml:512 * k + 128 * (ml + 1)],
                                wv_sb[:, 256 * k:256 * (k + 1)],
                                start=(k == 0), stop=(k == 5))
                        v4 = v2_sb[tp].rearrange('p (k h c) -> p k h c', k=2, h=4)
                        nc.vector.tensor_copy(
                            v4[:, half, :, 0:64],
                            pv[:].rearrange('p (h d) -> p h d', h=4))

                acc = {}
                av_pending = []  # deferred AV matmuls (software pipeline)
                AV_LAG = 2

                def av_drain(keep):
                    while len(av_pending) > keep:
                        av_pending.pop(0)()

                def strip_pair(sc, p, tp):
                    # both heads of pair p for m tiles 2tp,2tp+1; QK issued
                    # head-interleaved so base-0/base-64 matmuls overlap on
                    # the two 64-row halves of the PE array
                    if tp == 0:
                        for a in range(2):
                            acc[(p, a)] = psAV.tile([68, 512], F32,
                                                    tag=f'acc{a}', name=f'acc{a}')
                    tabs = [psQK.tile([128, 1024], F32, tag='tAB',
                                      name=f'tab{a}') for a in range(2)]
                    kt = kt_sb[p][tp // 2]
                    base = 256 * (tp % 2)
                    for half in range(2):
                        for a in range(2):
                            nc.tensor.matmul(
                                tabs[a][:, 512 * half:512 * (half + 1)],
                                kt[:, base + 128 * half:base + 128 * (half + 1)],
                                qt_z[p][a][:, SC_W * sc:SC_W * (sc + 1)],
                                start=True, stop=True)
                    for a in range(2):
                        at = apool.tile([128, 1024], FP8, tag='at')
                        nc.scalar.activation(at[:], tabs[a][:], AF.Exp,
                                             scale=float(Dh) ** -0.5)
                        a_cc = acc[(p, a)]

                        def av(a_cc=a_cc, at=at, tp=tp, p=p, a=a):
                            nc.tensor.matmul(
                                a_cc[:],
                                v2_sb[tp].rearrange('p (k h c) -> p k h c',
                                                    k=2, h=4)[:, :, 2 * p + a, :],
                                at[:].rearrange('p (k s) -> p k s', k=2),
                                start=(tp == 0), stop=(tp == N_TP - 1),
                                perf_mode=DR)
                        av_pending.append(av)
                    av_drain(AV_LAG)

                def rt_send(sc, p):
                    av_drain(0)
                    for a in range(2):
                        rt_t = rtst.tile([66, 512], BF16, tag='rt_t')
                        nc.vector.tensor_copy(rt_t[:], acc.pop((p, a))[0:66, :])
                        nc.sync.dma_start(
                            out=rt_in[(sc, p)][66 * a:66 * (a + 1), :],
                            in_=rt_t[:])
                    nc.gpsimd.collective_compute(
                        'AllGather', ALU.bypass,
                        replica_groups=[list(range(NC))],
                        ins=[rt_in[(sc, p)][:].opt()],
                        outs=[rt_out[(sc, p)][:].opt()])

                def pgq_step(dt):
                    # Wg1 @ q part, own 512 columns, output chunk dt
                    pgq = psKV.tile([128, 512], F32, tag='pk')
                    for kc in range(8):
                        nc.tensor.matmul(
                            pgq[:],
                            wg1_sb[:, DM * kc + 128 * dt:DM * kc + 128 * (dt + 1)],
                            qs_sb[:, 512 * kc:512 * (kc + 1)],
                            start=(kc == 0), stop=(kc == 7))
                    nc.vector.tensor_copy(pgq_sb[:, 512 * dt:512 * (dt + 1)], pgq[:])

                # --- epilogue phase over a pair of s-chunks (256 cols) ---
                # steps split by head-pair parity: kc-even chunks depend only
                # on the (sc, p=0) gathers, kc-odd on (sc, p=1)
                def e_steps(scs):
                    c0 = EC * scs[0]  # column offset into the 512 own cols
                    W = EC * len(scs)

                    def cols(t, dt):
                        return t[:, 512 * dt + c0:512 * dt + c0 + W]

                    rtn = {}
                    rec = {}

                    def e_denom(par):
                        def f():
                            dgt = ept.tile([16, 256], BF16, tag='dgt')
                            rdf = ept.tile([16, 256], F32, tag='rdf')
                            rc = ept.tile([16, 256], F32R, tag=f'rec{par}',
                                          name=f'rec{par}')
                            for i, sc in enumerate(scs):
                                nc.gpsimd.indirect_dma_start(
                                    out=dgt[:, 128 * i:128 * (i + 1)],
                                    out_offset=None, in_=rt_out[(sc, par)][:],
                                    in_offset=bass.IndirectOffsetOnAxis(
                                        ap=didx_sb[:], axis=0))
                            nc.vector.reciprocal(rdf[:, 0:W], dgt[:, 0:W])
                            nc.vector.tensor_copy(rc[:, 0:W], rdf[:, 0:W])
                            rec[par] = rc
                        return f

                    def e_rtn(kc):
                        def f():
                            raw = ept.tile([128, 256], BF16, tag='raw')
                            for i, sc in enumerate(scs):
                                nc.gpsimd.indirect_dma_start(
                                    out=raw[:, 128 * i:128 * (i + 1)],
                                    out_offset=None, in_=rt_out[(sc, kc % 2)][:],
                                    in_offset=bass.IndirectOffsetOnAxis(
                                        ap=gidx_sb[kc][:], axis=0))
                            bcp = psKV.tile([128, 256], F32, tag='pv')
                            nc.tensor.matmul(
                                bcp[:, 0:W], sel16_sb[:, 128 * kc:128 * (kc + 1)],
                                rec[kc % 2][:, 0:W], start=True, stop=True)
                            rt = rtnp.tile([128, 256], BF16, tag=f'rtn{kc}',
                                           name=f'rtn{kc}')
                            nc.vector.tensor_tensor(rt[:, 0:W], raw[:, 0:W],
                                                    bcp[:, 0:W], ALU.mult)
                            rtn[kc] = rt
                        return f

                    def e_wo(dt):
                        def f():
                            po = psKV.tile([128, 512], F32, tag='pk')
                            for i, kc in enumerate([0, 2, 4, 6, 1, 3, 5, 7]):
                                nc.tensor.matmul(
                                    po[:, 0:W],
                                    wo_sb[:, DM * kc + 128 * dt:DM * kc + 128 * (dt + 1)],
                                    rtn[kc][:, 0:W], start=(i == 0), stop=(i == 7))
                            nc.vector.tensor_scalar_add(
                                cols(oT, dt), po[:, 0:W], bo2_sb[:, dt:dt + 1])
                        return f

                    def e_g1(dt):
                        def f():
                            pg = psKV.tile([128, 512], F32, tag='pk')
                            for kc in range(8):
                                nc.tensor.matmul(
                                    pg[:, 0:W],
                                    wg1_sb[:, DM * (8 + kc) + 128 * dt:DM * (8 + kc) + 128 * (dt + 1)],
                                    cols(oT, kc), start=(kc == 0), stop=(kc == 7))
                            gsum = ept.tile([128, 256], F32, tag='gsum')
                            nc.vector.tensor_tensor(gsum[:, 0:W], pg[:, 0:W],
                                                    cols(pgq_sb, dt), ALU.add)
                            sg = ept.tile([128, 256], F32, tag='sg')
                            nc.scalar.activation(sg[:, 0:W], gsum[:, 0:W],
                                                 AF.Sigmoid,
                                                 bias=bg1_sb[:, dt:dt + 1])
                            gg = ept.tile([128, 256], F32, tag='gg')
                            nc.vector.tensor_scalar_add(gg[:, 0:W], gsum[:, 0:W],
                                                        bg1_sb[:, dt:dt + 1])
                            nc.vector.tensor_tensor(cols(sl, dt), gg[:, 0:W],
                                                    sg[:, 0:W], ALU.mult)
                        return f

                    gbs_ref = {}

                    def e_gate():
                        pgt_t = psKV.tile([128, 256], F32, tag='pv')
                        pgt = pgt_t[0:2, 0:W]
                        for kc in range(8):
                            nc.tensor.matmul(pgt, wg2_sb[:, 2 * kc:2 * (kc + 1)],
                                             cols(sl, kc),
                                             start=(kc == 0), stop=(kc == 7))
                        gate = ept.tile([2, 256], F32R, tag='gate')
                        nc.scalar.activation(gate[:, 0:W], pgt, AF.Sigmoid,
                                             bias=bg2_sb[:])
                        gb = psKV.tile([128, 256], F32, tag='pv')
                        nc.tensor.matmul(gb[:, 0:W], bc0_sb[:], gate[:, 0:W],
                                         start=True, stop=True)
                        gbs = ept.tile([128, 256], F32, tag='gbs')
                        nc.vector.tensor_copy(gbs[:, 0:W], gb[:, 0:W])
                        gbs_ref[0] = gbs

                    def e_out(dt):
                        def f():
                            go = ept.tile([128, 256], F32, tag='go')
                            nc.vector.tensor_tensor(go[:, 0:W], gbs_ref[0][:, 0:W],
                                                    cols(oT, dt), ALU.mult)
                            fo = ept.tile([128, 256], F32, tag='fo')
                            nc.vector.tensor_tensor(fo[:, 0:W], go[:, 0:W],
                                                    cols(qs_sb, dt), ALU.add)
                            for i, sc in enumerate(scs):
                                nc.sync.dma_start(
                                    out=out_t[128 * dt:128 * (dt + 1),
                                              128 * sc:128 * (sc + 1)],
                                    in_=fo[:, 128 * i:128 * (i + 1)])
                        return f

                    return {
                        'even': [e_denom(0)] + [e_rtn(kc) for kc in (0, 2, 4, 6)],
                        'odd': [e_denom(1)] + [e_rtn(kc) for kc in (1, 3, 5, 7)],
                        'wo': [e_wo(dt) for dt in range(8)],
                        'g1': [e_g1(dt) for dt in range(8)] + [e_gate],
                        'out': [e_out(dt) for dt in range(8)],
                    }

                def window(sc, p, fills, fill_start=4):
                    # 16 strip-pairs with fill steps spread from fill_start on
                    # (fills that depend on an AllGather issued just before
                    # this window need ~20us of strips queued ahead of them)
                    slots = {}
                    n = len(fills)
                    span = 15 - fill_start
                    for i in range(n):
                        slots.setdefault(
                            fill_start + (span * i) // max(n - 1, 1),
                            []).append(fills[i])
                    for tp in range(N_TP):
                        strip_pair(sc, p, tp)
                        for f in slots.get(tp, []):
                            f()
                    rt_send(sc, p)

                # ---------------- the weave ----------------
                kv_dma(0)
                kv_dma(1)
                qproj_j(0)
                kv_block(0)
                kv_dma(2)
                kv_block(1)
                # sc0/p0 strips interleaved with remaining KV blocks; first
                # strips go as early as possible to start the exp pipeline
                for mc in range(2, 8):
                    strip_pair(0, 0, 2 * (mc - 2))
                    strip_pair(0, 0, 2 * mc - 3)
                    if mc < 7:
                        kv_dma(mc + 1)
                    kv_block(mc)
                for tp in (12, 13, 14, 15):
                    strip_pair(0, 0, tp)
                rt_send(0, 0)
                load_epilogue_weights()

                ea = e_steps((0, 1))
                e2 = e_steps((2,))
                e3 = e_steps((3,))
                pgq = [lambda d=d: pgq_step(d) for d in range(8)]
                window(0, 1, [lambda j=j: qproj_j(j) for j in (1, 2, 3)])
                window(1, 0, pgq[0:4])
                window(1, 1, pgq[4:8])
                window(2, 0, ea['even'], fill_start=13)
                window(2, 1, ea['odd'] + ea['wo'][0:2], fill_start=6)
                window(3, 0, ea['wo'][2:8] + ea['g1'] + ea['out'])
                window(3, 1, e2['even'] + e2['odd'] + e2['wo'] + e2['g1'],
                       fill_start=4)
                # tail: sc2 outputs (no AG dep) and sc3-even hide under the
                # final AllGather's latency; only sc3-odd onward is serial
                for f in e2['out'] + e3['even']:
                    f()
                for f in e3['odd'] + e3['wo'] + e3['g1'] + e3['out']:
                    f()

            ctx_psKV.__exit__(None, None, None)
            ctx_mkin.__exit__(None, None, None)

    nc.compile()
    return nc


def _shard(inputs):
    import ml_dtypes
    bf16 = ml_dtypes.bfloat16
    f8 = ml_dtypes.float8_e4m3fn
    q = np.asarray(inputs['query'], np.float32)
    mk = np.asarray(inputs['memory_keys'], np.float32)
    mv = np.asarray(inputs['memory_values'], np.float32)
    Wq = np.asarray(inputs['Wq'], np.float32); bq = np.asarray(inputs['bq'], np.float32)
    Wk = np.asarray(inputs['Wk'], np.float32); bk = np.asarray(inputs['bk'], np.float32)
    Wv = np.asarray(inputs['Wv'], np.float32); bv = np.asarray(inputs['bv'], np.float32)
    Wo = np.asarray(inputs['Wo'], np.float32); bo = np.asarray(inputs['bo'], np.float32)
    Wg1 = np.asarray(inputs['Wg1'], np.float32); bg1 = np.asarray(inputs['bg1'], np.float32)
    Wg2 = np.asarray(inputs['Wg2'], np.float32); bg2 = np.asarray(inputs['bg2'], np.float32)

    scale = Dh ** -0.5
    bo2 = bo + Wo @ bv
    wg2T = np.zeros((DM, 2), np.float32)
    wg2T[:, 0] = Wg2[0]
    bg2v = np.zeros((2, 1), np.float32)
    bg2v[:, 0] = bg2[0]
    bc0 = np.zeros((2, 128), np.float32)
    bc0[0, :] = 1.0
    # sel16[h, 128*kc + j] = 1 where head(2kc + j//64) == h
    sel16 = np.zeros((16, 1024), np.float32)
    for kc in range(8):
        for j in range(128):
            sel16[2 * kc + j // 64, 128 * kc + j] = 1.0
    vones = np.tile(np.array([1.0, 0.0, 0.0, 0.0], np.float32), 8)[None, :].repeat(128, 0)

    qT_b = [np.ascontiguousarray(q[b].T) for b in range(B)]
    mkT_b = [np.ascontiguousarray(mk[b].T) for b in range(B)]
    mvT_b = [np.ascontiguousarray(mv[b].T) for b in range(B)]

    in_maps = []
    for c in range(NC):
        b, g = c // GS, c % GS
        hs = slice(64 * 4 * g, 64 * (4 * g + 4))
        # own epilogue columns: s = 512*sc + 128*g + i  -> qsT col 128*sc + i
        own_s = np.concatenate(
            [np.arange(512 * sc + 128 * g, 512 * sc + 128 * g + 128)
             for sc in range(N_SC)])
        # gather row indices in the flat [4224, 128] per-(sc,p) view:
        # flat = 4*row + g, row(h, d) = 132*(4b + h//4) + 66*(h%2) + d
        gidx_rows = np.zeros((8, 128), np.int32)
        for kc in range(8):
            for j in range(128):
                r = 132 * (4 * b + kc // 2) + 66 * (j // 64) + (j % 64)
                gidx_rows[kc, j] = 4 * r + g
        didx_rows = np.zeros((1, 16), np.int32)
        for h in range(16):
            r = 132 * (4 * b + h // 4) + 66 * (h % 2) + 64
            didx_rows[0, h] = 4 * r + g
        in_maps.append({
            'qT': qT_b[b].astype(f8),
            'mkT': mkT_b[b].astype(f8),
            'mvT': mvT_b[b].astype(f8),
            'wqT': np.ascontiguousarray(Wq[hs].T).astype(f8),
            'wkT': np.ascontiguousarray(Wk[hs].T).astype(f8),
            'wvT': np.ascontiguousarray(Wv[hs].T).astype(f8),
            'woT': np.ascontiguousarray(Wo.T).astype(bf16),
            'wg1T': np.ascontiguousarray(Wg1.T).astype(bf16),
            'wg2T': wg2T.astype(bf16),
            'qsT': np.ascontiguousarray(q[b].T[:, own_s]).astype(bf16),
            'bqv': np.ascontiguousarray(bq[hs].reshape(2, 128)),
            'bkv': np.ascontiguousarray(bk[hs].reshape(2, 128)),
            'bo2v': np.ascontiguousarray(bo2.reshape(8, 128)),
            'bg1v': np.ascontiguousarray(bg1.reshape(8, 128)),
            'bg2v': bg2v,
            'sel16': sel16,
            'bc0': bc0,
            'vones': vones.astype(f8),
            'gidx': gidx_rows,
            'didx': didx_rows,
        })
    return in_maps


def _run(inputs, trace=False):
    global _PROG
    from concourse.bass_utils import run_bass_kernel_spmd
    if _PROG is None:
        _PROG = _build_program()
    in_maps = _shard(inputs)
    res = run_bass_kernel_spmd(_PROG, in_maps, list(range(NC)), trace=trace)
    out = np.empty((B, S, DM), np.float32)
    for c in range(NC):
        b, g = c // GS, c % GS
        o = res.results[c]['out_t']  # [DM, 512], cols = 128*sc + i
        for sc in range(N_SC):
            out[b, 512 * sc + 128 * g:512 * sc + 128 * (g + 1), :] = \
                o[:, 128 * sc:128 * (sc + 1)].T
    return out, res


def kernel(**inputs) -> np.ndarray:
    out, _ = _run(inputs, trace=False)
    return out
